# revision 11
# baseline (speedup 1.0000x reference)
"""External Attention (nn_External_Attention) on 8 TRN2 NeuronCores.

kernel(x, Wk, Wv) -> x + Wv @ l1norm_M(softmax_N(Wk @ x))
  x  [16, 512, 4096] f32,  Wk [256, 512] f32,  Wv [512, 256] f32

Sharding: data-parallel over batch B=16 -> 2 batches per core across 8 cores.

v2c design notes (on top of v2b's stage-batched anti-convoy structure):
  - All I/O bf16; bf16 matmuls; y written in place into the x tiles.
  - Lead-in: packed weight DMAs (1 issue each), batch-0 x loaded via 4
    kc-spanning 1MB quarter DMAs, batch-1 x via one 4MB DMA; 16 warmup
    matmuls on a memset tile keep the PE HAM-warm through the load window.
  - Normalizer chain per batch, stage-batched: cs matmuls ([1,1024] PSUM)
    -> DVE reciprocal_approx_fast -> ACT bf16 cast -> GPSIMD partition
    broadcast (the only GPSIMD op type -> its Q7 library loads once).
  - E' = E * bc on DVE at [128,2048] (2x bf16 mode).
  - Consume loop co-major with j-paired matmuls (stationary operand reused
    back-to-back); y stores per co-half so the last store is only 512KB.
  - Residual adds alternate: even tiles direct DVE (PSUM f32 + x -> bf16),
    odd tiles ACT copy to bf16 + DVE bf16 add.
"""
from contextlib import ExitStack

import ml_dtypes
import numpy as np

import concourse.bacc as bacc
import concourse.mybir as mybir
import concourse.tile as tile
from concourse.bass_utils import run_bass_kernel_spmd

F32 = mybir.dt.float32
BF16 = mybir.dt.bfloat16
AF = mybir.ActivationFunctionType
ALU = mybir.AluOpType
AX = mybir.AxisListType

BF16NP = ml_dtypes.bfloat16

B, C, M, N = 16, 512, 256, 4096
NCORES = 8
BPC = B // NCORES
KC = C // 128   # 4 c-blocks
KM = M // 128   # 2 m-blocks
NT = 512        # matmul tile width
NJ = N // NT    # 8
XT = 1024       # exp / cs tile width
NJ2 = N // XT   # 4
WT = 2048       # E' / bc tile width
NW = N // WT    # 2
NWARM = 16


def _build(nc):
    x_d = nc.dram_tensor("x", [BPC, C, N], BF16, kind="ExternalInput").ap()
    wkT_d = nc.dram_tensor("wkT", [C, M], BF16, kind="ExternalInput").ap()
    wvT_d = nc.dram_tensor("wvT", [M, C], BF16, kind="ExternalInput").ap()
    y_d = nc.dram_tensor("y", [BPC, C, N], BF16, kind="ExternalOutput").ap()
    # DRAM scratch for the per-column normalizers: lets a later DMA
    # partition-broadcast them (0-stride partition APs are DRAM-only).
    rcs_d = nc.dram_tensor("rcs_scratch", [BPC, N], F32, kind="Internal").ap()

    with tile.TileContext(nc) as tc, ExitStack() as ctx:
        wpool = ctx.enter_context(tc.tile_pool(name="w", bufs=1))
        xpool = ctx.enter_context(tc.tile_pool(name="xp", bufs=2))
        epool = ctx.enter_context(tc.tile_pool(name="ep", bufs=2 * KM))
        spool = ctx.enter_context(tc.tile_pool(name="sp", bufs=10))
        wvp_pool = ctx.enter_context(tc.tile_pool(name="wvp", bufs=2 * KM))
        rcpool = ctx.enter_context(tc.tile_pool(name="rc", bufs=4))
        bcpool = ctx.enter_context(tc.tile_pool(name="bcp", bufs=2 * NW))
        eppool = ctx.enter_context(tc.tile_pool(name="epp", bufs=2 * KM))
        evpool = ctx.enter_context(tc.tile_pool(name="ev", bufs=6))
        ps_l = ctx.enter_context(tc.tile_pool(name="ps_l", bufs=2, space="PSUM"))
        ps_cs = ctx.enter_context(tc.tile_pool(name="ps_cs", bufs=1, space="PSUM"))
        ps_o = ctx.enter_context(tc.tile_pool(name="ps_o", bufs=2, space="PSUM"))

        # PE warmup: 16 matmuls on a zeroed tile keep HAM busy while x loads.
        wz = wpool.tile([128, NT], BF16, tag="wz", name="wz")
        nc.vector.memset(wz[:], 0)
        for i in range(NWARM):
            po = ps_o.tile([128, NT], F32, tag="po", name=f"warm{i}")
            nc.tensor.matmul(po[:], wz[:, 0:128], wz[:], start=True, stop=True)

        # packed weight loads: one DMA each
        wk_sb = wpool.tile([128, KC * M], BF16, tag="wk", name="wk")
        nc.sync.dma_start(wk_sb[:].rearrange("p (k m) -> p k m", k=KC),
                          wkT_d.rearrange("(k p) m -> p k m", k=KC))
        wv_sb = wpool.tile([128, KM * C], BF16, tag="wv", name="wv")
        nc.scalar.dma_start(wv_sb[:].rearrange("p (k m) -> p k m", k=KM),
                            wvT_d.rearrange("(k p) m -> p k m", k=KM))

        def wk_ap(kc, km):
            return wk_sb[:, kc * M + km * 128: kc * M + (km + 1) * 128]

        X, E, RSP, RRB, WVP, CS, RCS, BC, EP = ({} for _ in range(9))
        ev_idx = [0]

        # x tiles: one [128, KC*N] tile per batch; kc block at [:, kc*N:+N]
        for b in range(BPC):
            X[b] = xpool.tile([128, KC * N], BF16, tag="x", name=f"x{b}")
        src0 = x_d[0].rearrange("(k p) n -> p k n", k=KC)
        dst0 = X[0][:].rearrange("p (k n) -> p k n", k=KC)
        q = N // 4
        for h in range(4):
            nc.sync.dma_start(dst0[:, :, h * q:(h + 1) * q],
                              src0[:, :, h * q:(h + 1) * q])
        nc.sync.dma_start(X[1][:].rearrange("p (k n) -> p k n", k=KC),
                          x_d[1].rearrange("(k p) n -> p k n", k=KC))

        def xs(b, kc, sl):
            return X[b][:, kc * N + sl.start: kc * N + sl.stop]

        def init_batch(b):
            E[b] = [epool.tile([128, N], BF16, tag="e", name=f"e{b}_{km}")
                    for km in range(KM)]
            RSP[b] = [spool.tile([128, NJ2], F32, tag="rsp", name=f"rsp{b}_{km}")
                      for km in range(KM)]
            EP[b] = [eppool.tile([128, N], BF16, tag="epp", name=f"epp{b}_{km}")
                     for km in range(KM)]
            CS[b], RCS[b], BC[b] = [], [], []

        def emit_A(b, jj):
            # MM1 + exp for columns [jj*XT, (jj+1)*XT); kc outer so the
            # stationary operand is reused across the two h halves.
            for km in range(KM):
                pl = ps_l.tile([128, XT], F32, tag="pl", name=f"pl{b}_{jj}_{km}")
                for kc in range(KC):
                    for h in range(XT // NT):
                        nc.tensor.matmul(
                            pl[:, h * NT:(h + 1) * NT],
                            wk_ap(kc, km),
                            xs(b, kc, slice(jj * XT + h * NT,
                                            jj * XT + (h + 1) * NT)),
                            start=(kc == 0), stop=(kc == KC - 1))
                nc.scalar.activation(
                    E[b][km][:, jj * XT:(jj + 1) * XT], pl[:],
                    AF.Exp, accum_out=RSP[b][km][:, jj:jj + 1])

        def emit_stats(b):
            RRB[b], WVP[b] = [], []
            for km in range(KM):
                rs = spool.tile([128, 1], F32, tag="rs", name=f"rs{b}_{km}")
                nc.vector.tensor_reduce(rs[:], RSP[b][km][:], axis=AX.X, op=ALU.add)
                rr = spool.tile([128, 1], F32, tag="rr", name=f"rr{b}_{km}")
                nc.vector.reciprocal(rr[:], rs[:])
                rrb = spool.tile([128, 1], BF16, tag="rrb", name=f"rrb{b}_{km}")
                nc.vector.tensor_copy(rrb[:], rr[:])
                RRB[b].append(rrb)
                t = wvp_pool.tile([128, C], BF16, tag="wvp", name=f"wvp{b}_{km}")
                nc.vector.tensor_scalar_mul(t[:], wv_sb[:, km * C:(km + 1) * C], rr[:])
                WVP[b].append(t)

        def emit_cs(b, j2):
            # colsum for columns [j2*XT, (j2+1)*XT) + reciprocal
            cs = ps_cs.tile([1, XT], F32, tag="cs", name=f"cs{b}_{j2}")
            for km in range(KM):
                for h in range(XT // NT):
                    nc.tensor.matmul(
                        cs[:, h * NT:(h + 1) * NT], RRB[b][km][:],
                        E[b][km][:, j2 * XT + h * NT: j2 * XT + (h + 1) * NT],
                        start=(km == 0), stop=(km == KM - 1))
            rcs = rcpool.tile([1, XT], F32, tag="rcs", name=f"rcs{b}_{j2}")
            nc.vector.reciprocal_approx_fast(rcs[:], cs[:])
            nc.sync.dma_start(rcs_d[b, j2 * XT:(j2 + 1) * XT].unsqueeze(0), rcs[:])
            RCS[b].append(rcs)

        def emit_bcast(b, w):
            # partition-broadcast + f32->bf16 cast in one SWDGE DMA from the
            # DRAM scratch: a 0-stride partition dim replicates the row
            # across all 128 partitions, the cast happens in the datapath.
            bc = bcpool.tile([128, WT], BF16, tag="bc", name=f"bc{b}_{w}")
            src = rcs_d[b, w * WT:(w + 1) * WT].partition_broadcast(128)
            nc.gpsimd.dma_start(bc[:], src)
            BC[b].append(bc)

        def emit_epmul(b, w):
            sl = slice(w * WT, (w + 1) * WT)
            for km in range(KM):
                nc.gpsimd.tensor_tensor(EP[b][km][:, sl], E[b][km][:, sl],
                                        BC[b][w][:], op=ALU.mult)

        def emit_consume_co(b, co):
            # all 8 j tiles of one co block: MM2 (j-paired, km outer for
            # stationary reuse) + residual add; stores per co-half
            for jp in range(NJ // 2):
                pos = [ps_o.tile([128, NT], F32, tag="po", name=f"po{b}_{co}_{j}")
                       for j in (2 * jp, 2 * jp + 1)]
                for km in range(KM):
                    for k, j in enumerate((2 * jp, 2 * jp + 1)):
                        nc.tensor.matmul(
                            pos[k][:],
                            WVP[b][km][:, co * 128:(co + 1) * 128],
                            EP[b][km][:, j * NT:(j + 1) * NT],
                            start=(km == 0), stop=(km == KM - 1))
                for k, j in enumerate((2 * jp, 2 * jp + 1)):
                    sl = slice(j * NT, (j + 1) * NT)
                    ys = xs(b, co, sl)
                    r = ev_idx[0] % 16
                    if r in (0, 2, 4, 6, 8, 10, 12, 13, 14):
                        nc.vector.tensor_tensor(ys, pos[k][:], ys, op=ALU.add)
                    else:
                        t = evpool.tile([128, NT], BF16, tag="ev",
                                        name=f"ev{b}_{co}_{j}")
                        nc.scalar.activation(t[:], pos[k][:], AF.Copy)
                        if r in (1, 5, 9, 15):
                            nc.gpsimd.tensor_tensor(ys, t[:], ys, op=ALU.add)
                        else:
                            nc.vector.tensor_tensor(ys, t[:], ys, op=ALU.add)
                    ev_idx[0] += 1
                if jp == 1 or jp == 3:
                    h = jp // 2
                    nc.sync.dma_start(
                        y_d[b, co * 128:(co + 1) * 128, h * (N // 2):(h + 1) * (N // 2)],
                        xs(b, co, slice(h * (N // 2), (h + 1) * (N // 2))))

        # ---- program ----
        init_batch(0)
        init_batch(1)
        for jj in range(NJ2):
            emit_A(0, jj)
        emit_stats(0)
        for j2 in range(NJ2):
            emit_cs(0, j2)
            emit_A(1, j2)
        for w in range(NW):
            emit_bcast(0, w)
            emit_epmul(0, w)
        emit_consume_co(0, 0)
        emit_stats(1)
        emit_cs(1, 0)
        emit_consume_co(0, 1)
        emit_cs(1, 1)
        emit_consume_co(0, 2)
        emit_cs(1, 2)
        emit_consume_co(0, 3)
        emit_cs(1, 3)
        for w in range(NW):
            emit_bcast(1, w)
            emit_epmul(1, w)
        for co in range(KC):
            emit_consume_co(1, co)
    return nc


_CACHE = {}


def _get_program():
    if "nc" not in _CACHE:
        nc = bacc.Bacc("TRN2", target_bir_lowering=False, debug=False,
                       enable_asserts=True)
        _build(nc)
        nc.compile()
        _CACHE["nc"] = nc
    return _CACHE["nc"]


def _in_maps(x, Wk, Wv):
    x = np.asarray(x, dtype=np.float32)
    xb = np.ascontiguousarray(x).astype(BF16NP)
    wkT = np.ascontiguousarray(np.asarray(Wk, np.float32).T).astype(BF16NP)
    wvT = np.ascontiguousarray(np.asarray(Wv, np.float32).T).astype(BF16NP)
    return [{"x": xb[i * BPC:(i + 1) * BPC], "wkT": wkT, "wvT": wvT}
            for i in range(NCORES)]


def kernel(x, Wk, Wv):
    nc = _get_program()
    res = run_bass_kernel_spmd(nc, _in_maps(x, Wk, Wv), list(range(NCORES)))
    y = np.concatenate([res.results[i]["y"].astype(np.float32)
                        for i in range(NCORES)], axis=0)
    return np.ascontiguousarray(y)


# revision 13
# speedup vs baseline: 1.2847x; 1.2847x over previous
"""External Attention (nn_External_Attention) on 8 TRN2 NeuronCores.

kernel(x, Wk, Wv) -> x + Wv @ l1norm_M(softmax_N(Wk @ x))
  x  [16, 512, 4096] f32,  Wk [256, 512] f32,  Wv [512, 256] f32

Sharding: data-parallel over batch B=16 -> 2 batches per core across 8 cores.

v2c design notes (on top of v2b's stage-batched anti-convoy structure):
  - All I/O bf16; bf16 matmuls; y written in place into the x tiles.
  - Lead-in: packed weight DMAs (1 issue each), batch-0 x loaded via 4
    kc-spanning 1MB quarter DMAs, batch-1 x via one 4MB DMA; 16 warmup
    matmuls on a memset tile keep the PE HAM-warm through the load window.
  - Normalizer chain per batch, stage-batched: cs matmuls ([1,1024] PSUM)
    -> DVE reciprocal_approx_fast -> ACT bf16 cast -> GPSIMD partition
    broadcast (the only GPSIMD op type -> its Q7 library loads once).
  - E' = E * bc on DVE at [128,2048] (2x bf16 mode).
  - Consume loop co-major with j-paired matmuls (stationary operand reused
    back-to-back); y stores per co-half so the last store is only 512KB.
  - Residual adds alternate: even tiles direct DVE (PSUM f32 + x -> bf16),
    odd tiles ACT copy to bf16 + DVE bf16 add.
"""
from contextlib import ExitStack

import ml_dtypes
import numpy as np

import concourse.bacc as bacc
import concourse.mybir as mybir
import concourse.tile as tile
from concourse.bass_utils import run_bass_kernel_spmd

F32 = mybir.dt.float32
BF16 = mybir.dt.bfloat16
AF = mybir.ActivationFunctionType
ALU = mybir.AluOpType
AX = mybir.AxisListType

BF16NP = ml_dtypes.bfloat16

B, C, M, N = 16, 512, 256, 4096
NCORES = 8
BPC = B // NCORES
KC = C // 128   # 4 c-blocks
KM = M // 128   # 2 m-blocks
NT = 512        # matmul tile width
NJ = N // NT    # 8
XT = 1024       # exp / cs tile width
NJ2 = N // XT   # 4
WT = 2048       # E' / bc tile width
NW = N // WT    # 2
NWARM = 16


def _build(nc):
    x_d = nc.dram_tensor("x", [BPC, C, N], BF16, kind="ExternalInput").ap()
    wkT_d = nc.dram_tensor("wkT", [C, M], BF16, kind="ExternalInput").ap()
    wvT_d = nc.dram_tensor("wvT", [M, C], BF16, kind="ExternalInput").ap()
    y_d = nc.dram_tensor("y", [BPC, C, N], BF16, kind="ExternalOutput").ap()
    # DRAM scratch for the per-column normalizers: lets a later DMA
    # partition-broadcast them (0-stride partition APs are DRAM-only).
    rcs_d = nc.dram_tensor("rcs_scratch", [BPC, N], F32, kind="Internal").ap()

    with tile.TileContext(nc) as tc, ExitStack() as ctx:
        wpool = ctx.enter_context(tc.tile_pool(name="w", bufs=1))
        xpool = ctx.enter_context(tc.tile_pool(name="xp", bufs=2))
        epool = ctx.enter_context(tc.tile_pool(name="ep", bufs=2 * KM))
        spool = ctx.enter_context(tc.tile_pool(name="sp", bufs=10))
        wvp_pool = ctx.enter_context(tc.tile_pool(name="wvp", bufs=2 * KM))
        rcpool = ctx.enter_context(tc.tile_pool(name="rc", bufs=4))
        bcpool = ctx.enter_context(tc.tile_pool(name="bcp", bufs=2 * NW))
        eppool = ctx.enter_context(tc.tile_pool(name="epp", bufs=2 * KM))
        evpool = ctx.enter_context(tc.tile_pool(name="ev", bufs=6))
        ps_l = ctx.enter_context(tc.tile_pool(name="ps_l", bufs=2, space="PSUM"))
        ps_cs = ctx.enter_context(tc.tile_pool(name="ps_cs", bufs=1, space="PSUM"))
        ps_o = ctx.enter_context(tc.tile_pool(name="ps_o", bufs=2, space="PSUM"))

        # PE warmup: 16 matmuls on a zeroed tile keep HAM busy while x loads.
        wz = wpool.tile([128, NT], BF16, tag="wz", name="wz")
        nc.vector.memset(wz[:], 0)
        for i in range(NWARM):
            po = ps_o.tile([128, NT], F32, tag="po", name=f"warm{i}")
            nc.tensor.matmul(po[:], wz[:, 0:128], wz[:], start=True, stop=True)

        # packed weight loads: one DMA each
        wk_sb = wpool.tile([128, KC * M], BF16, tag="wk", name="wk")
        nc.sync.dma_start(wk_sb[:].rearrange("p (k m) -> p k m", k=KC),
                          wkT_d.rearrange("(k p) m -> p k m", k=KC))
        wv_sb = wpool.tile([128, KM * C], BF16, tag="wv", name="wv")
        nc.scalar.dma_start(wv_sb[:].rearrange("p (k m) -> p k m", k=KM),
                            wvT_d.rearrange("(k p) m -> p k m", k=KM))

        def wk_ap(kc, km):
            return wk_sb[:, kc * M + km * 128: kc * M + (km + 1) * 128]

        X, E, RSP, RRB, WVP, CS, RCS, BC, EP = ({} for _ in range(9))
        ev_idx = [0]

        # x tiles: one [128, KC*N] tile per batch; kc block at [:, kc*N:+N]
        for b in range(BPC):
            X[b] = xpool.tile([128, KC * N], BF16, tag="x", name=f"x{b}")
        src0 = x_d[0].rearrange("(k p) n -> p k n", k=KC)
        dst0 = X[0][:].rearrange("p (k n) -> p k n", k=KC)
        q = N // 4
        for h in range(4):
            nc.sync.dma_start(dst0[:, :, h * q:(h + 1) * q],
                              src0[:, :, h * q:(h + 1) * q])
        nc.sync.dma_start(X[1][:].rearrange("p (k n) -> p k n", k=KC),
                          x_d[1].rearrange("(k p) n -> p k n", k=KC))

        def xs(b, kc, sl):
            return X[b][:, kc * N + sl.start: kc * N + sl.stop]

        def init_batch(b):
            E[b] = [epool.tile([128, N], BF16, tag="e", name=f"e{b}_{km}")
                    for km in range(KM)]
            RSP[b] = [spool.tile([128, NJ2], F32, tag="rsp", name=f"rsp{b}_{km}")
                      for km in range(KM)]
            EP[b] = [eppool.tile([128, N], BF16, tag="epp", name=f"epp{b}_{km}")
                     for km in range(KM)]
            CS[b], RCS[b], BC[b] = [], [], []

        def emit_A(b, jj):
            # MM1 + exp for columns [jj*XT, (jj+1)*XT); kc outer so the
            # stationary operand is reused across the two h halves.
            for km in range(KM):
                pl = ps_l.tile([128, XT], F32, tag="pl", name=f"pl{b}_{jj}_{km}")
                for kc in range(KC):
                    for h in range(XT // NT):
                        nc.tensor.matmul(
                            pl[:, h * NT:(h + 1) * NT],
                            wk_ap(kc, km),
                            xs(b, kc, slice(jj * XT + h * NT,
                                            jj * XT + (h + 1) * NT)),
                            start=(kc == 0), stop=(kc == KC - 1))
                nc.scalar.activation(
                    E[b][km][:, jj * XT:(jj + 1) * XT], pl[:],
                    AF.Exp, accum_out=RSP[b][km][:, jj:jj + 1])

        def emit_stats(b):
            RRB[b], WVP[b] = [], []
            for km in range(KM):
                rs = spool.tile([128, 1], F32, tag="rs", name=f"rs{b}_{km}")
                nc.vector.tensor_reduce(rs[:], RSP[b][km][:], axis=AX.X, op=ALU.add)
                rr = spool.tile([128, 1], F32, tag="rr", name=f"rr{b}_{km}")
                nc.vector.reciprocal(rr[:], rs[:])
                rrb = spool.tile([128, 1], BF16, tag="rrb", name=f"rrb{b}_{km}")
                nc.vector.tensor_copy(rrb[:], rr[:])
                RRB[b].append(rrb)
                t = wvp_pool.tile([128, C], BF16, tag="wvp", name=f"wvp{b}_{km}")
                nc.vector.tensor_scalar_mul(t[:], wv_sb[:, km * C:(km + 1) * C], rr[:])
                WVP[b].append(t)

        def emit_cs(b, j2):
            # colsum for columns [j2*XT, (j2+1)*XT) + reciprocal
            cs = ps_cs.tile([1, XT], F32, tag="cs", name=f"cs{b}_{j2}")
            for km in range(KM):
                for h in range(XT // NT):
                    nc.tensor.matmul(
                        cs[:, h * NT:(h + 1) * NT], RRB[b][km][:],
                        E[b][km][:, j2 * XT + h * NT: j2 * XT + (h + 1) * NT],
                        start=(km == 0), stop=(km == KM - 1))
            rcs = rcpool.tile([1, XT], F32, tag="rcs", name=f"rcs{b}_{j2}")
            nc.vector.reciprocal_approx_fast(rcs[:], cs[:])
            nc.sync.dma_start(rcs_d[b, j2 * XT:(j2 + 1) * XT].unsqueeze(0), rcs[:])
            RCS[b].append(rcs)

        def emit_bcast(b, w):
            # partition-broadcast + f32->bf16 cast in one SWDGE DMA from the
            # DRAM scratch: a 0-stride partition dim replicates the row
            # across all 128 partitions, the cast happens in the datapath.
            bc = bcpool.tile([128, WT], BF16, tag="bc", name=f"bc{b}_{w}")
            src = rcs_d[b, w * WT:(w + 1) * WT].partition_broadcast(128)
            nc.gpsimd.dma_start(bc[:], src)
            BC[b].append(bc)

        def emit_epmul(b, w):
            sl = slice(w * WT, (w + 1) * WT)
            for km in range(KM):
                nc.vector.tensor_tensor(EP[b][km][:, sl], E[b][km][:, sl],
                                        BC[b][w][:], op=ALU.mult)

        def emit_consume_co(b, co):
            # all 8 j tiles of one co block: MM2 (j-paired, km outer for
            # stationary reuse) + residual add; stores per co-half
            for jp in range(NJ // 2):
                pos = [ps_o.tile([128, NT], F32, tag="po", name=f"po{b}_{co}_{j}")
                       for j in (2 * jp, 2 * jp + 1)]
                for km in range(KM):
                    for k, j in enumerate((2 * jp, 2 * jp + 1)):
                        nc.tensor.matmul(
                            pos[k][:],
                            WVP[b][km][:, co * 128:(co + 1) * 128],
                            EP[b][km][:, j * NT:(j + 1) * NT],
                            start=(km == 0), stop=(km == KM - 1))
                for k, j in enumerate((2 * jp, 2 * jp + 1)):
                    sl = slice(j * NT, (j + 1) * NT)
                    ys = xs(b, co, sl)
                    r = ev_idx[0] % 16
                    if r % 2 == 0:
                        nc.vector.tensor_tensor(ys, pos[k][:], ys, op=ALU.add)
                    else:
                        t = evpool.tile([128, NT], BF16, tag="ev",
                                        name=f"ev{b}_{co}_{j}")
                        nc.scalar.activation(t[:], pos[k][:], AF.Copy)
                        if r in (13, 15):
                            nc.vector.tensor_tensor(ys, t[:], ys, op=ALU.add)
                        else:
                            nc.gpsimd.tensor_tensor(ys, t[:], ys, op=ALU.add)
                    ev_idx[0] += 1
                if jp == 1 or jp == 3:
                    h = jp // 2
                    nc.sync.dma_start(
                        y_d[b, co * 128:(co + 1) * 128, h * (N // 2):(h + 1) * (N // 2)],
                        xs(b, co, slice(h * (N // 2), (h + 1) * (N // 2))))

        # ---- program ----
        init_batch(0)
        init_batch(1)
        for jj in range(NJ2):
            emit_A(0, jj)
        emit_stats(0)
        for j2 in range(NJ2):
            emit_cs(0, j2)
            emit_A(1, j2)
        for w in range(NW):
            emit_bcast(0, w)
            emit_epmul(0, w)
        emit_consume_co(0, 0)
        emit_stats(1)
        emit_cs(1, 0)
        emit_consume_co(0, 1)
        emit_cs(1, 1)
        emit_consume_co(0, 2)
        emit_cs(1, 2)
        emit_consume_co(0, 3)
        emit_cs(1, 3)
        for w in range(NW):
            emit_bcast(1, w)
            emit_epmul(1, w)
        for co in range(KC):
            emit_consume_co(1, co)
    return nc


_CACHE = {}


def _get_program():
    if "nc" not in _CACHE:
        nc = bacc.Bacc("TRN2", target_bir_lowering=False, debug=False,
                       enable_asserts=True)
        _build(nc)
        nc.compile()
        _CACHE["nc"] = nc
    return _CACHE["nc"]


def _in_maps(x, Wk, Wv):
    x = np.asarray(x, dtype=np.float32)
    xb = np.ascontiguousarray(x).astype(BF16NP)
    wkT = np.ascontiguousarray(np.asarray(Wk, np.float32).T).astype(BF16NP)
    wvT = np.ascontiguousarray(np.asarray(Wv, np.float32).T).astype(BF16NP)
    return [{"x": xb[i * BPC:(i + 1) * BPC], "wkT": wkT, "wvT": wvT}
            for i in range(NCORES)]


def kernel(x, Wk, Wv):
    nc = _get_program()
    res = run_bass_kernel_spmd(nc, _in_maps(x, Wk, Wv), list(range(NCORES)))
    y = np.concatenate([res.results[i]["y"].astype(np.float32)
                        for i in range(NCORES)], axis=0)
    return np.ascontiguousarray(y)


# revision 14
# speedup vs baseline: 1.3314x; 1.0363x over previous
"""External Attention (nn_External_Attention) on 8 TRN2 NeuronCores.

kernel(x, Wk, Wv) -> x + Wv @ l1norm_M(softmax_N(Wk @ x))
  x  [16, 512, 4096] f32,  Wk [256, 512] f32,  Wv [512, 256] f32

Sharding: data-parallel over batch B=16 -> 2 batches per core across 8 cores.

v2c design notes (on top of v2b's stage-batched anti-convoy structure):
  - All I/O bf16; bf16 matmuls; y written in place into the x tiles.
  - Lead-in: packed weight DMAs (1 issue each), batch-0 x loaded via 4
    kc-spanning 1MB quarter DMAs, batch-1 x via one 4MB DMA; 16 warmup
    matmuls on a memset tile keep the PE HAM-warm through the load window.
  - Normalizer chain per batch, stage-batched: cs matmuls ([1,1024] PSUM)
    -> DVE reciprocal_approx_fast -> ACT bf16 cast -> GPSIMD partition
    broadcast (the only GPSIMD op type -> its Q7 library loads once).
  - E' = E * bc on DVE at [128,2048] (2x bf16 mode).
  - Consume loop co-major with j-paired matmuls (stationary operand reused
    back-to-back); y stores per co-half so the last store is only 512KB.
  - Residual adds alternate: even tiles direct DVE (PSUM f32 + x -> bf16),
    odd tiles ACT copy to bf16 + DVE bf16 add.
"""
from contextlib import ExitStack

import ml_dtypes
import numpy as np

import concourse.bacc as bacc
import concourse.mybir as mybir
import concourse.tile as tile
from concourse.bass_utils import run_bass_kernel_spmd

F32 = mybir.dt.float32
BF16 = mybir.dt.bfloat16
AF = mybir.ActivationFunctionType
ALU = mybir.AluOpType
AX = mybir.AxisListType

BF16NP = ml_dtypes.bfloat16

B, C, M, N = 16, 512, 256, 4096
NCORES = 8
BPC = B // NCORES
KC = C // 128   # 4 c-blocks
KM = M // 128   # 2 m-blocks
NT = 512        # matmul tile width
NJ = N // NT    # 8
XT = 1024       # exp / cs tile width
NJ2 = N // XT   # 4
WT = 2048       # E' / bc tile width
NW = N // WT    # 2
NWARM = 16


def _build(nc):
    x_d = nc.dram_tensor("x", [BPC, C, N], BF16, kind="ExternalInput").ap()
    wkT_d = nc.dram_tensor("wkT", [C, M], BF16, kind="ExternalInput").ap()
    wvT_d = nc.dram_tensor("wvT", [M, C], BF16, kind="ExternalInput").ap()
    y_d = nc.dram_tensor("y", [BPC, C, N], BF16, kind="ExternalOutput").ap()
    # DRAM scratch for the per-column normalizers: lets a later DMA
    # partition-broadcast them (0-stride partition APs are DRAM-only).
    rcs_d = nc.dram_tensor("rcs_scratch", [BPC, N], F32, kind="Internal").ap()

    with tile.TileContext(nc) as tc, ExitStack() as ctx:
        wpool = ctx.enter_context(tc.tile_pool(name="w", bufs=1))
        xpool = ctx.enter_context(tc.tile_pool(name="xp", bufs=2))
        epool = ctx.enter_context(tc.tile_pool(name="ep", bufs=2 * KM))
        spool = ctx.enter_context(tc.tile_pool(name="sp", bufs=10))
        wvp_pool = ctx.enter_context(tc.tile_pool(name="wvp", bufs=2 * KM))
        rcpool = ctx.enter_context(tc.tile_pool(name="rc", bufs=4))
        bcpool = ctx.enter_context(tc.tile_pool(name="bcp", bufs=2 * NW))
        eppool = ctx.enter_context(tc.tile_pool(name="epp", bufs=2 * KM))
        evpool = ctx.enter_context(tc.tile_pool(name="ev", bufs=6))
        ps_l = ctx.enter_context(tc.tile_pool(name="ps_l", bufs=2, space="PSUM"))
        ps_cs = ctx.enter_context(tc.tile_pool(name="ps_cs", bufs=2, space="PSUM"))
        ps_o = ctx.enter_context(tc.tile_pool(name="ps_o", bufs=4, space="PSUM"))

        # PE warmup: 16 matmuls on a zeroed tile keep HAM busy while x loads.
        wz = wpool.tile([128, NT], BF16, tag="wz", name="wz")
        nc.vector.memset(wz[:], 0)
        for i in range(NWARM):
            po = ps_o.tile([128, NT], F32, tag="po", name=f"warm{i}")
            nc.tensor.matmul(po[:], wz[:, 0:128], wz[:], start=True, stop=True)

        # packed weight loads: one DMA each
        wk_sb = wpool.tile([128, KC * M], BF16, tag="wk", name="wk")
        nc.sync.dma_start(wk_sb[:].rearrange("p (k m) -> p k m", k=KC),
                          wkT_d.rearrange("(k p) m -> p k m", k=KC))
        wv_sb = wpool.tile([128, KM * C], BF16, tag="wv", name="wv")
        nc.scalar.dma_start(wv_sb[:].rearrange("p (k m) -> p k m", k=KM),
                            wvT_d.rearrange("(k p) m -> p k m", k=KM))

        def wk_ap(kc, km):
            return wk_sb[:, kc * M + km * 128: kc * M + (km + 1) * 128]

        X, E, RSP, RRB, WVP, CS, RCS, BC, EP = ({} for _ in range(9))
        ev_idx = [0]

        # x tiles: one [128, KC*N] tile per batch; kc block at [:, kc*N:+N]
        for b in range(BPC):
            X[b] = xpool.tile([128, KC * N], BF16, tag="x", name=f"x{b}")
        src0 = x_d[0].rearrange("(k p) n -> p k n", k=KC)
        dst0 = X[0][:].rearrange("p (k n) -> p k n", k=KC)
        q = N // 4
        for h in range(4):
            nc.sync.dma_start(dst0[:, :, h * q:(h + 1) * q],
                              src0[:, :, h * q:(h + 1) * q])
        nc.sync.dma_start(X[1][:].rearrange("p (k n) -> p k n", k=KC),
                          x_d[1].rearrange("(k p) n -> p k n", k=KC))

        def xs(b, kc, sl):
            return X[b][:, kc * N + sl.start: kc * N + sl.stop]

        def init_batch(b):
            E[b] = [epool.tile([128, N], BF16, tag="e", name=f"e{b}_{km}")
                    for km in range(KM)]
            RSP[b] = [spool.tile([128, NJ], F32, tag="rsp", name=f"rsp{b}_{km}")
                      for km in range(KM)]
            EP[b] = [eppool.tile([128, N], BF16, tag="epp", name=f"epp{b}_{km}")
                     for km in range(KM)]
            CS[b], RCS[b], BC[b] = [], [], []

        def emit_A(b, jj):
            # MM1 + exp for columns [jj*XT, (jj+1)*XT), in 512-wide units
            for km in range(KM):
                for h in range(XT // NT):
                    j = jj * (XT // NT) + h
                    pl = ps_l.tile([128, NT], F32, tag="pl", name=f"pl{b}_{j}_{km}")
                    for kc in range(KC):
                        nc.tensor.matmul(
                            pl[:], wk_ap(kc, km),
                            xs(b, kc, slice(j * NT, (j + 1) * NT)),
                            start=(kc == 0), stop=(kc == KC - 1))
                    nc.scalar.activation(
                        E[b][km][:, j * NT:(j + 1) * NT], pl[:],
                        AF.Exp, accum_out=RSP[b][km][:, j:j + 1])

        def emit_stats(b):
            RRB[b], WVP[b] = [], []
            for km in range(KM):
                rs = spool.tile([128, 1], F32, tag="rs", name=f"rs{b}_{km}")
                nc.vector.tensor_reduce(rs[:], RSP[b][km][:], axis=AX.X, op=ALU.add)
                rr = spool.tile([128, 1], F32, tag="rr", name=f"rr{b}_{km}")
                nc.vector.reciprocal(rr[:], rs[:])
                rrb = spool.tile([128, 1], BF16, tag="rrb", name=f"rrb{b}_{km}")
                nc.vector.tensor_copy(rrb[:], rr[:])
                RRB[b].append(rrb)
                t = wvp_pool.tile([128, C], BF16, tag="wvp", name=f"wvp{b}_{km}")
                nc.vector.tensor_scalar_mul(t[:], wv_sb[:, km * C:(km + 1) * C], rr[:])
                WVP[b].append(t)

        def emit_cs(b, j2):
            # colsum + reciprocal for columns [j2*XT, (j2+1)*XT), 512-wide
            for h in range(XT // NT):
                j = j2 * (XT // NT) + h
                cs = ps_cs.tile([1, NT], F32, tag="cs", name=f"cs{b}_{j}")
                for km in range(KM):
                    nc.tensor.matmul(
                        cs[:], RRB[b][km][:],
                        E[b][km][:, j * NT:(j + 1) * NT],
                        start=(km == 0), stop=(km == KM - 1))
                rcs = rcpool.tile([1, NT], F32, tag="rcs", name=f"rcs{b}_{j}")
                nc.vector.reciprocal_approx_fast(rcs[:], cs[:])
                nc.sync.dma_start(rcs_d[b, j * NT:(j + 1) * NT].unsqueeze(0), rcs[:])
                RCS[b].append(rcs)

        def emit_bcast(b, w):
            # partition-broadcast + f32->bf16 cast in one SWDGE DMA from the
            # DRAM scratch: a 0-stride partition dim replicates the row
            # across all 128 partitions, the cast happens in the datapath.
            bc = bcpool.tile([128, WT], BF16, tag="bc", name=f"bc{b}_{w}")
            src = rcs_d[b, w * WT:(w + 1) * WT].partition_broadcast(128)
            nc.gpsimd.dma_start(bc[:], src)
            BC[b].append(bc)

        def emit_epmul(b, w):
            sl = slice(w * WT, (w + 1) * WT)
            for km in range(KM):
                nc.vector.tensor_tensor(EP[b][km][:, sl], E[b][km][:, sl],
                                        BC[b][w][:], op=ALU.mult)

        def emit_consume_co(b, co):
            # all 8 j tiles of one co block: MM2 (j-paired, km outer for
            # stationary reuse) + residual add; stores per co-half
            for jp in range(NJ // 2):
                pos = [ps_o.tile([128, NT], F32, tag="po", name=f"po{b}_{co}_{j}")
                       for j in (2 * jp, 2 * jp + 1)]
                for km in range(KM):
                    for k, j in enumerate((2 * jp, 2 * jp + 1)):
                        nc.tensor.matmul(
                            pos[k][:],
                            WVP[b][km][:, co * 128:(co + 1) * 128],
                            EP[b][km][:, j * NT:(j + 1) * NT],
                            start=(km == 0), stop=(km == KM - 1))
                for k, j in enumerate((2 * jp, 2 * jp + 1)):
                    sl = slice(j * NT, (j + 1) * NT)
                    ys = xs(b, co, sl)
                    r = ev_idx[0] % 16
                    if r % 2 == 0:
                        nc.vector.tensor_tensor(ys, pos[k][:], ys, op=ALU.add)
                    else:
                        t = evpool.tile([128, NT], BF16, tag="ev",
                                        name=f"ev{b}_{co}_{j}")
                        nc.scalar.activation(t[:], pos[k][:], AF.Copy)
                        if r in (13, 15):
                            nc.vector.tensor_tensor(ys, t[:], ys, op=ALU.add)
                        else:
                            nc.gpsimd.tensor_tensor(ys, t[:], ys, op=ALU.add)
                    ev_idx[0] += 1
                if jp == 1 or jp == 3:
                    h = jp // 2
                    nc.sync.dma_start(
                        y_d[b, co * 128:(co + 1) * 128, h * (N // 2):(h + 1) * (N // 2)],
                        xs(b, co, slice(h * (N // 2), (h + 1) * (N // 2))))

        # ---- program ----
        init_batch(0)
        init_batch(1)
        for jj in range(NJ2):
            emit_A(0, jj)
        emit_stats(0)
        for j2 in range(NJ2):
            emit_cs(0, j2)
            emit_A(1, j2)
        for w in range(NW):
            emit_bcast(0, w)
            emit_epmul(0, w)
        emit_consume_co(0, 0)
        emit_stats(1)
        emit_cs(1, 0)
        emit_consume_co(0, 1)
        emit_cs(1, 1)
        emit_consume_co(0, 2)
        emit_cs(1, 2)
        emit_consume_co(0, 3)
        emit_cs(1, 3)
        for w in range(NW):
            emit_bcast(1, w)
            emit_epmul(1, w)
        for co in range(KC):
            emit_consume_co(1, co)
    return nc


_CACHE = {}


def _get_program():
    if "nc" not in _CACHE:
        nc = bacc.Bacc("TRN2", target_bir_lowering=False, debug=False,
                       enable_asserts=True)
        _build(nc)
        nc.compile()
        _CACHE["nc"] = nc
    return _CACHE["nc"]


def _in_maps(x, Wk, Wv):
    x = np.asarray(x, dtype=np.float32)
    xb = np.ascontiguousarray(x).astype(BF16NP)
    wkT = np.ascontiguousarray(np.asarray(Wk, np.float32).T).astype(BF16NP)
    wvT = np.ascontiguousarray(np.asarray(Wv, np.float32).T).astype(BF16NP)
    return [{"x": xb[i * BPC:(i + 1) * BPC], "wkT": wkT, "wvT": wvT}
            for i in range(NCORES)]


def kernel(x, Wk, Wv):
    nc = _get_program()
    res = run_bass_kernel_spmd(nc, _in_maps(x, Wk, Wv), list(range(NCORES)))
    y = np.concatenate([res.results[i]["y"].astype(np.float32)
                        for i in range(NCORES)], axis=0)
    return np.ascontiguousarray(y)


# revision 16
# speedup vs baseline: 1.4947x; 1.1227x over previous
"""External Attention (nn_External_Attention) on 8 TRN2 NeuronCores.

kernel(x, Wk, Wv) -> x + Wv @ l1norm_M(softmax_N(Wk @ x))
  x  [16, 512, 4096] f32,  Wk [256, 512] f32,  Wv [512, 256] f32

Sharding: data-parallel over batch B=16 -> 2 batches per core across 8 cores.

v2c design notes (on top of v2b's stage-batched anti-convoy structure):
  - All I/O bf16; bf16 matmuls; y written in place into the x tiles.
  - Lead-in: packed weight DMAs (1 issue each), batch-0 x loaded via 4
    kc-spanning 1MB quarter DMAs, batch-1 x via one 4MB DMA; 16 warmup
    matmuls on a memset tile keep the PE HAM-warm through the load window.
  - Normalizer chain per batch, stage-batched: cs matmuls ([1,1024] PSUM)
    -> DVE reciprocal_approx_fast -> ACT bf16 cast -> GPSIMD partition
    broadcast (the only GPSIMD op type -> its Q7 library loads once).
  - E' = E * bc on DVE at [128,2048] (2x bf16 mode).
  - Consume loop co-major with j-paired matmuls (stationary operand reused
    back-to-back); y stores per co-half so the last store is only 512KB.
  - Residual adds alternate: even tiles direct DVE (PSUM f32 + x -> bf16),
    odd tiles ACT copy to bf16 + DVE bf16 add.
"""
from contextlib import ExitStack

import ml_dtypes
import numpy as np

import concourse.bacc as bacc
import concourse.mybir as mybir
import concourse.tile as tile
from concourse.bass_utils import run_bass_kernel_spmd

F32 = mybir.dt.float32
BF16 = mybir.dt.bfloat16
AF = mybir.ActivationFunctionType
ALU = mybir.AluOpType
AX = mybir.AxisListType

BF16NP = ml_dtypes.bfloat16

B, C, M, N = 16, 512, 256, 4096
NCORES = 8
BPC = B // NCORES
KC = C // 128   # 4 c-blocks
KM = M // 128   # 2 m-blocks
NT = 512        # matmul tile width
NJ = N // NT    # 8
XT = 1024       # exp / cs tile width
NJ2 = N // XT   # 4
WT = 2048       # E' / bc tile width
NW = N // WT    # 2
NWARM = 16


def _build(nc):
    x_d = nc.dram_tensor("x", [BPC, C, N], BF16, kind="ExternalInput").ap()
    wkT_d = nc.dram_tensor("wkT", [C, M], BF16, kind="ExternalInput").ap()
    wvT_d = nc.dram_tensor("wvT", [M, C], BF16, kind="ExternalInput").ap()
    y_d = nc.dram_tensor("y", [BPC, C, N], BF16, kind="ExternalOutput").ap()
    # DRAM scratch for the per-column normalizers: lets a later DMA
    # partition-broadcast them (0-stride partition APs are DRAM-only).
    rcs_d = nc.dram_tensor("rcs_scratch", [BPC, N], F32, kind="Internal").ap()

    with tile.TileContext(nc) as tc, ExitStack() as ctx:
        wpool = ctx.enter_context(tc.tile_pool(name="w", bufs=1))
        xpool = ctx.enter_context(tc.tile_pool(name="xp", bufs=2))
        epool = ctx.enter_context(tc.tile_pool(name="ep", bufs=2 * KM))
        spool = ctx.enter_context(tc.tile_pool(name="sp", bufs=10))
        wvp_pool = ctx.enter_context(tc.tile_pool(name="wvp", bufs=2 * KM))
        rcpool = ctx.enter_context(tc.tile_pool(name="rc", bufs=1))
        bcpool = ctx.enter_context(tc.tile_pool(name="bcp", bufs=3))
        eppool = ctx.enter_context(tc.tile_pool(name="epp", bufs=2 * KM))
        evpool = ctx.enter_context(tc.tile_pool(name="ev", bufs=4))
        ps_pp = ctx.enter_context(tc.tile_pool(name="ps_pp", bufs=3, space="PSUM"))
        ps_cs = ctx.enter_context(tc.tile_pool(name="ps_cs", bufs=2, space="PSUM"))

        # PE warmup: 16 matmuls on a zeroed tile keep HAM busy while x loads.
        wz = wpool.tile([128, NT], BF16, tag="wz", name="wz")
        nc.vector.memset(wz[:], 0)
        for i in range(NWARM):
            po = ps_pp.tile([128, XT], F32, tag="pp", name=f"warm{i}")
            nc.tensor.matmul(po[:, 0:NT], wz[:, 0:128], wz[:], start=True, stop=True)

        # packed weight loads: one DMA each
        wk_sb = wpool.tile([128, KC * M], BF16, tag="wk", name="wk")
        nc.sync.dma_start(wk_sb[:].rearrange("p (k m) -> p k m", k=KC),
                          wkT_d.rearrange("(k p) m -> p k m", k=KC))
        wv_sb = wpool.tile([128, KM * C], BF16, tag="wv", name="wv")
        nc.scalar.dma_start(wv_sb[:].rearrange("p (k m) -> p k m", k=KM),
                            wvT_d.rearrange("(k p) m -> p k m", k=KM))

        def wk_ap(kc, km):
            return wk_sb[:, kc * M + km * 128: kc * M + (km + 1) * 128]

        X, E, RSP, RRB, WVP, CS, RCS, BC, EP = ({} for _ in range(9))
        ev_idx = [0]

        # x tiles: one [128, KC*N] tile per batch; kc block at [:, kc*N:+N]
        for b in range(BPC):
            X[b] = xpool.tile([128, KC * N], BF16, tag="x", name=f"x{b}")
        src0 = x_d[0].rearrange("(k p) n -> p k n", k=KC)
        dst0 = X[0][:].rearrange("p (k n) -> p k n", k=KC)
        q = N // 4
        for h in range(4):
            nc.sync.dma_start(dst0[:, :, h * q:(h + 1) * q],
                              src0[:, :, h * q:(h + 1) * q])
        nc.sync.dma_start(X[1][:].rearrange("p (k n) -> p k n", k=KC),
                          x_d[1].rearrange("(k p) n -> p k n", k=KC))

        def xs(b, kc, sl):
            return X[b][:, kc * N + sl.start: kc * N + sl.stop]

        def init_batch(b):
            RCS[b] = rcpool.tile([1, N], F32, tag="rcsall", name=f"rcsall{b}")
            E[b] = [epool.tile([128, N], BF16, tag="e", name=f"e{b}_{km}")
                    for km in range(KM)]
            RSP[b] = [spool.tile([128, NJ2], F32, tag="rsp", name=f"rsp{b}_{km}")
                      for km in range(KM)]
            EP[b] = [eppool.tile([128, N], BF16, tag="epp", name=f"epp{b}_{km}")
                     for km in range(KM)]
            CS[b], BC[b] = [], []

        def emit_A(b, jj):
            # MM1 + exp for columns [jj*XT, (jj+1)*XT); kc outer so the
            # stationary operand is reused across the two h halves
            for km in range(KM):
                pl = ps_pp.tile([128, XT], F32, tag="pp", name=f"pl{b}_{jj}_{km}")
                for kc in range(KC):
                    for h in range(XT // NT):
                        nc.tensor.matmul(
                            pl[:, h * NT:(h + 1) * NT], wk_ap(kc, km),
                            xs(b, kc, slice(jj * XT + h * NT,
                                            jj * XT + (h + 1) * NT)),
                            start=(kc == 0), stop=(kc == KC - 1))
                nc.scalar.activation(
                    E[b][km][:, jj * XT:(jj + 1) * XT], pl[:],
                    AF.Exp, accum_out=RSP[b][km][:, jj:jj + 1])

        def emit_stats(b):
            RRB[b], WVP[b] = [], []
            for km in range(KM):
                rs = spool.tile([128, 1], F32, tag="rs", name=f"rs{b}_{km}")
                nc.vector.tensor_reduce(rs[:], RSP[b][km][:], axis=AX.X, op=ALU.add)
                rr = spool.tile([128, 1], F32, tag="rr", name=f"rr{b}_{km}")
                nc.vector.reciprocal(rr[:], rs[:])
                rrb = spool.tile([128, 1], BF16, tag="rrb", name=f"rrb{b}_{km}")
                nc.vector.tensor_copy(rrb[:], rr[:])
                RRB[b].append(rrb)
                t = wvp_pool.tile([128, C], BF16, tag="wvp", name=f"wvp{b}_{km}")
                nc.vector.tensor_scalar_mul(t[:], wv_sb[:, km * C:(km + 1) * C], rr[:])
                WVP[b].append(t)

        def emit_cs(b, j2):
            # colsum + reciprocal for columns [j2*XT, (j2+1)*XT), 512-wide
            for h in range(XT // NT):
                j = j2 * (XT // NT) + h
                cs = ps_cs.tile([1, NT], F32, tag="cs", name=f"cs{b}_{j}")
                for km in range(KM):
                    nc.tensor.matmul(
                        cs[:], RRB[b][km][:],
                        E[b][km][:, j * NT:(j + 1) * NT],
                        start=(km == 0), stop=(km == KM - 1))
                nc.vector.reciprocal_approx_fast(
                    RCS[b][:, j * NT:(j + 1) * NT], cs[:])

        def emit_rcs_store(b, w):
            # SWDGE store (gpsimd queue: never blocked behind y-stores)
            nc.gpsimd.dma_start(rcs_d[b, w * WT:(w + 1) * WT].unsqueeze(0),
                                RCS[b][:, w * WT:(w + 1) * WT])

        def emit_bcast(b, w):
            # partition-broadcast + f32->bf16 cast in one SWDGE DMA from the
            # DRAM scratch: a 0-stride partition dim replicates the row
            # across all 128 partitions, the cast happens in the datapath.
            bc = bcpool.tile([128, WT], BF16, tag="bc", name=f"bc{b}_{w}")
            src = rcs_d[b, w * WT:(w + 1) * WT].partition_broadcast(128)
            nc.gpsimd.dma_start(bc[:], src)
            BC[b].append(bc)

        def emit_epmul(b, w):
            sl = slice(w * WT, (w + 1) * WT)
            for km in range(KM):
                nc.vector.tensor_tensor(EP[b][km][:, sl], E[b][km][:, sl],
                                        BC[b][w][:], op=ALU.mult)

        def emit_consume_co(b, co):
            # all 8 j tiles of one co block: MM2 (j-paired, km outer for
            # stationary reuse) + residual add; stores per co-half
            for jp in range(NJ // 2):
                pp = ps_pp.tile([128, XT], F32, tag="pp", name=f"po{b}_{co}_{jp}")
                for km in range(KM):
                    for k, j in enumerate((2 * jp, 2 * jp + 1)):
                        nc.tensor.matmul(
                            pp[:, k * NT:(k + 1) * NT],
                            WVP[b][km][:, co * 128:(co + 1) * 128],
                            EP[b][km][:, j * NT:(j + 1) * NT],
                            start=(km == 0), stop=(km == KM - 1))
                for k, j in enumerate((2 * jp, 2 * jp + 1)):
                    po = pp[:, k * NT:(k + 1) * NT]
                    sl = slice(j * NT, (j + 1) * NT)
                    ys = xs(b, co, sl)
                    r = ev_idx[0] % 16
                    if r % 2 == 0:
                        nc.vector.tensor_tensor(ys, po, ys, op=ALU.add)
                    else:
                        t = evpool.tile([128, NT], BF16, tag="ev",
                                        name=f"ev{b}_{co}_{j}")
                        nc.scalar.activation(t[:], po, AF.Copy)
                        if r % 4 == 3:
                            nc.vector.tensor_tensor(ys, t[:], ys, op=ALU.add)
                        else:
                            nc.gpsimd.tensor_tensor(ys, t[:], ys, op=ALU.add)
                    ev_idx[0] += 1
                if jp == 1 or jp == 3:
                    h = jp // 2
                    nc.sync.dma_start(
                        y_d[b, co * 128:(co + 1) * 128, h * (N // 2):(h + 1) * (N // 2)],
                        xs(b, co, slice(h * (N // 2), (h + 1) * (N // 2))))

        # ---- program ----
        init_batch(0)
        init_batch(1)
        for jj in range(NJ2):
            emit_A(0, jj)
        emit_stats(0)
        emit_cs(0, 0)
        emit_A(1, 0)
        emit_cs(0, 1)
        emit_A(1, 1)
        emit_rcs_store(0, 0)
        emit_bcast(0, 0)
        emit_epmul(0, 0)
        emit_cs(0, 2)
        emit_A(1, 2)
        emit_cs(0, 3)
        emit_A(1, 3)
        emit_rcs_store(0, 1)
        emit_bcast(0, 1)
        emit_epmul(0, 1)
        emit_consume_co(0, 0)
        emit_stats(1)
        emit_cs(1, 0)
        emit_cs(1, 1)
        emit_consume_co(0, 1)
        emit_rcs_store(1, 0)
        emit_bcast(1, 0)
        emit_epmul(1, 0)
        emit_cs(1, 2)
        emit_cs(1, 3)
        emit_consume_co(0, 2)
        emit_rcs_store(1, 1)
        emit_bcast(1, 1)
        emit_epmul(1, 1)
        emit_consume_co(0, 3)
        for co in range(KC):
            emit_consume_co(1, co)
    return nc


_CACHE = {}


def _get_program():
    if "nc" not in _CACHE:
        nc = bacc.Bacc("TRN2", target_bir_lowering=False, debug=False,
                       enable_asserts=True)
        _build(nc)
        nc.compile()
        _CACHE["nc"] = nc
    return _CACHE["nc"]


def _in_maps(x, Wk, Wv):
    x = np.asarray(x, dtype=np.float32)
    xb = np.ascontiguousarray(x).astype(BF16NP)
    wkT = np.ascontiguousarray(np.asarray(Wk, np.float32).T).astype(BF16NP)
    wvT = np.ascontiguousarray(np.asarray(Wv, np.float32).T).astype(BF16NP)
    return [{"x": xb[i * BPC:(i + 1) * BPC], "wkT": wkT, "wvT": wvT}
            for i in range(NCORES)]


def kernel(x, Wk, Wv):
    nc = _get_program()
    res = run_bass_kernel_spmd(nc, _in_maps(x, Wk, Wv), list(range(NCORES)))
    y = np.concatenate([res.results[i]["y"].astype(np.float32)
                        for i in range(NCORES)], axis=0)
    return np.ascontiguousarray(y)
